# revision 12
# baseline (speedup 1.0000x reference)
"""Distributed Trainium2 kernel for nn_AttentionBlock (channel attention).

Algorithm (exact algebra, no approximation):
  The attention matrix is [C,C] with the contraction over N=H*W*D tokens.
  GroupNorm is a per-channel affine xn = a*x + b whose stats derive from
  per-channel sums s = x@1 and the Gram matrix G = x@x.T (diag(G) = sumsq).
  Everything downstream of G is [C,C]-sized:
      S    = Wq' G Wk'^T + rank-1 terms        (Wq' = Wq diag(a))
      attn = softmax(S/sqrt(C))
      out  = x + P attn Wv' x + delta 1^T
  Pass 1 computes only the upper-triangle blocks of G (G is symmetric),
  with per-channel sums riding as a ones column in the same matmuls.
  The host pre-transposes/pre-casts x into fp16 token-major tiles so pass 1
  is pure matmul (no on-chip transposes), and pre-transposes the weights.
  The [C,C] partial results cross cores via an fp16 AllGather (the f32
  AllReduce ran at ~20GB/s bus here; a gather of the fp16 triangle + 3 DVE
  adds is ~4x faster), then the [C,C] chain and a streamed pass 2
      out = (I + A)^T x + delta
  with fp16 chunk loads and fully-contiguous fp16 block stores.

Sharding: batch B=2 x sequence 4  ->  8 cores. replica groups [[0..3],[4..7]].
"""

from contextlib import ExitStack

import numpy as np

import concourse.bass as bass
import concourse.tile as tile
from concourse import bacc, mybir
from concourse.bass_utils import run_bass_kernel_spmd
from concourse.bass import _add_dep_helper as add_dep

# Problem constants (hardcoded per harness contract)
B = 2
C = 512
N = 32768          # 32*32*32
NCORES = 8
SHARDS = 4         # sequence shards per batch
NS = N // SHARDS   # 8192 per-core tokens
GROUPS = 32
GSIZE = C // GROUPS  # 16
EPS = 1e-5
P = 128
CT = C // P        # 4 channel tiles
F32 = mybir.dt.float32
F16 = mybir.dt.float16

PH1_ITERS = NS // P             # 64 token chunks of 128
CPAD = C + 4                    # xt row: 512 cols + ones col + pad
XT_DMAS = 16
XT_STEP = PH1_ITERS // XT_DMAS  # 4 chunks per DMA
N_DUMMY = 160                   # PE keep-warm matmuls during the exchange

PH2_CHUNK = 512
PH2_STORE = 2048
N_STORES = NS // PH2_STORE      # 4
SUB = PH2_STORE // PH2_CHUNK    # 4

# triangle packing: row-block t holds blocks (t, t..3); blocks 1..3 carry a
# trailing s column (from the ones-col matmul); s0 is packed last.
TRI_W = [C - t * P for t in range(CT)]          # 512,384,256,128
TRI_OFF = [0, 512, 897, 1154]
PKW = 1284                                       # 512+385+257+129+1
S_COL = [1283, 896, 1153, 1282]                  # s cols for blocks 0..3

REPLICA_GROUPS = [[0, 1, 2, 3], [4, 5, 6, 7]]
SCALE = 1.0 / float(np.sqrt(C))


def build_graph():
    nc = bacc.Bacc(
        "TRN2", target_bir_lowering=False, debug=False, num_devices=NCORES
    )

    xt_ext = nc.dram_tensor("xt", [P, PH1_ITERS, CPAD], F16, kind="ExternalInput")
    xn_ext = nc.dram_tensor("xn", [P, N_STORES, CT, PH2_STORE], F16,
                            kind="ExternalInput")
    wqt_ext = nc.dram_tensor("wqt", [P, CT, C], F16, kind="ExternalInput")
    wkt_ext = nc.dram_tensor("wkt", [P, CT, C], F16, kind="ExternalInput")
    pwt_ext = nc.dram_tensor("pwt", [P, CT, C], F16, kind="ExternalInput")
    wv_ext = nc.dram_tensor("wv", [P, CT, C], F16, kind="ExternalInput")
    ident_ext = nc.dram_tensor("ident", [P, P], F32, kind="ExternalInput")
    adjc_ext = nc.dram_tensor("adjc", [P, P], F32, kind="ExternalInput")
    gn_w_ext = nc.dram_tensor("gn_w", [C], F32, kind="ExternalInput")
    gn_b_ext = nc.dram_tensor("gn_b", [C], F32, kind="ExternalInput")
    qkv_b_ext = nc.dram_tensor("qkv_b", [3 * C], F32, kind="ExternalInput")
    proj_b_ext = nc.dram_tensor("proj_b", [C], F32, kind="ExternalInput")
    out_ext = nc.dram_tensor("out", [P, N_STORES, CT, PH2_STORE], F16,
                             kind="ExternalOutput")

    with tile.TileContext(nc) as tc:
        _body(tc, xt_ext, xn_ext, wqt_ext, wkt_ext, pwt_ext, wv_ext,
              ident_ext, adjc_ext, gn_w_ext, gn_b_ext, qkv_b_ext,
              proj_b_ext, out_ext)

    nc.compile()
    return nc


def _body(tc, xt_ext, xn_ext, wqt_ext, wkt_ext, pwt_ext, wv_ext,
          ident_ext, adjc_ext, gn_w_ext, gn_b_ext, qkv_b_ext,
          proj_b_ext, out_ext):
    nc = tc.nc
    AX = mybir.AxisListType
    OP = mybir.AluOpType
    ACTF = mybir.ActivationFunctionType

    ctx = ExitStack()
    consts = ctx.enter_context(tc.tile_pool(name="consts", bufs=1))
    small = ctx.enter_context(tc.tile_pool(name="small", bufs=1))
    wpool = ctx.enter_context(tc.tile_pool(name="wpool", bufs=1))
    xt_pool = ctx.enter_context(tc.tile_pool(name="xt", bufs=1))
    chain = ctx.enter_context(tc.tile_pool(name="chain", bufs=1))
    xn_pool = ctx.enter_context(tc.tile_pool(name="xn", bufs=2))
    y_pool = ctx.enter_context(tc.tile_pool(name="yp", bufs=2))
    ps_g = ctx.enter_context(tc.tile_pool(name="psg", bufs=4, space="PSUM"))
    ps_t = ctx.enter_context(tc.tile_pool(name="pst", bufs=2, space="PSUM"))
    ps_y = ctx.enter_context(tc.tile_pool(name="psy", bufs=2, space="PSUM"))
    dram = ctx.enter_context(tc.tile_pool(name="dram", bufs=1, space="DRAM"))

    # ---------------- x token-major load (paces phase 1; full HWDGE rate,
    # nothing else touches HBM until these are done) ----------------
    xt_sb = xt_pool.tile([P, PH1_ITERS, CPAD], F16, name="xt_sb")
    xt_dmas = []
    for m in range(XT_DMAS):
        sl = slice(m * XT_STEP, (m + 1) * XT_STEP)
        xt_dmas.append(nc.sync.dma_start(xt_sb[:, sl], xt_ext[:, sl]))

    # small constants on the scalar (ACT) HWDGE ring
    ident = consts.tile([P, P], F32, name="ident")
    nc.scalar.dma_start(ident, ident_ext[:])
    adj = consts.tile([P, P], F32, name="adj")          # 16x16 block-diag ones
    nc.scalar.dma_start(adj, adjc_ext[:])
    gw_sb = consts.tile([P, CT], F32, name="gw_sb")
    gb_sb = consts.tile([P, CT], F32, name="gb_sb")
    pb_sb = consts.tile([P, CT], F32, name="pb_sb")
    nc.scalar.dma_start(gw_sb, gn_w_ext[:].rearrange("(t p) -> p t", p=P))
    nc.scalar.dma_start(gb_sb, gn_b_ext[:].rearrange("(t p) -> p t", p=P))
    nc.scalar.dma_start(pb_sb, proj_b_ext[:].rearrange("(t p) -> p t", p=P))
    qkvb_sb = consts.tile([P, 3 * CT], F32, name="qkvb_sb")
    nc.scalar.dma_start(qkvb_sb, qkv_b_ext[:].rearrange("(t p) -> p t", p=P))

    ident_h = consts.tile([P, P], F16, name="ident_h")
    nc.vector.tensor_copy(ident_h, ident)

    # ------- weights (pre-transposed on host; load after xt for HBM priority)
    WqT = wpool.tile([P, CT, C], F16, name="WqT")
    WkT = wpool.tile([P, CT, C], F16, name="WkT")
    PwT = wpool.tile([P, CT, C], F16, name="PwT")
    Wv_nat = wpool.tile([P, CT, C], F16, name="Wv_nat")
    w_dmas = []
    for Wdst, src in ((WqT, wqt_ext), (WkT, wkt_ext),
                      (PwT, pwt_ext), (Wv_nat, wv_ext)):
        w_dmas.append(nc.sync.dma_start(Wdst, src[:]))
    add_dep(w_dmas[0].ins, xt_dmas[-1].ins, sync=True,
            reason="weight loads must not contend with the xt load for HBM")

    # ------- phase 1: upper-triangle G blocks (fp16), s = x @ 1 -------
    # xt is token-major from the host: no transposes, just matmuls. The ones
    # column at xt col C makes s ride along for blocks 1..3; block 0's s
    # comes from a 1-col matmul reusing the block-0 stationary operand.
    G_ps = [ps_g.tile([P, TRI_W[t] + (1 if t > 0 else 0)], F32,
                      name=f"G_ps{t}", tag="g")
            for t in range(CT)]
    s0_ps = ps_t.tile([P, 1], F32, name="s0_ps", tag="pt")
    g_last = None
    for u in range(PH1_ITERS):
        xu = xt_sb[:, u]
        st, sp = (u == 0), (u == PH1_ITERS - 1)
        for t in range(CT):
            nc.tensor.matmul(
                G_ps[t],
                xu[:, t * P:(t + 1) * P],
                xu[:, t * P:C + (1 if t > 0 else 0)],
                start=st, stop=sp,
            )
        g_last = nc.tensor.matmul(
            s0_ps, xu[:, 0:P], xu[:, C:C + 1], start=st, stop=sp
        )

    # ---- pack fp16 payload [T0 | T1+s1 | T2+s2 | T3+s3 | s0] into our own
    # slot of the gather buffer ----
    g4 = chain.tile([P, SHARDS, PKW], F16, name="g4")
    pk = g4[:, 0]
    nc.scalar.copy(pk[:, 0:TRI_W[0]], G_ps[0])
    nc.vector.tensor_copy(pk[:, TRI_OFF[1]:TRI_OFF[1] + TRI_W[1] + 1], G_ps[1])
    nc.scalar.copy(pk[:, TRI_OFF[2]:TRI_OFF[2] + TRI_W[2] + 1], G_ps[2])
    nc.vector.tensor_copy(pk[:, TRI_OFF[3]:TRI_OFF[3] + TRI_W[3] + 1], G_ps[3])
    nc.vector.tensor_copy(pk[:, PKW - 1:PKW], s0_ps)

    # ------- PE keep-warm dummies: the HAM clock gate re-throttles the PE to
    # 1.2GHz after ~3.4us idle; the exchange window would otherwise make the
    # whole [C,C] chain run cold. Pinned after phase 1 on the PE queue. ------
    dummy_ps = ps_y.tile([P, C], F32, name="dummy_ps", tag="y")
    dmy_first = dmy_last = None
    for k in range(N_DUMMY):
        mm = nc.tensor.matmul(
            dummy_ps, xt_sb[:, 0, 0:P], xt_sb[:, 0, 0:C],
            start=True, stop=True,
        )
        if dmy_first is None:
            dmy_first = mm
        dmy_last = mm
    add_dep(dmy_first.ins, g_last.ins, sync=False,
            reason="dummies run after phase 1 in PE queue order")

    # ------- fp16 AllGather of the packed triangle (the f32 AllReduce ran at
    # 19.8GB/s bus; the gather moves half the bytes and skips the CCE reduce).
    # A hand-rolled remote_dma exchange was tried and is ~5x faster on paper,
    # but delivery is non-deterministic under this runtime shim — reverted.
    cc_in = dram.tile([P * PKW], F16, name="cc_in")
    cc_out = dram.tile([SHARDS * P * PKW], F16, name="cc_out")
    cc_iv = cc_in[:].rearrange("(p w) -> p w", p=P)
    # bounce-in in pieces so the DMA pipelines with the pack copies
    nc.scalar.dma_start(cc_iv[:, 0:TRI_OFF[2]], pk[:, 0:TRI_OFF[2]])
    nc.scalar.dma_start(cc_iv[:, TRI_OFF[2]:PKW], pk[:, TRI_OFF[2]:PKW])

    nc.gpsimd.collective_compute(
        "AllGather",
        OP.bypass,
        ins=[cc_in[:]],
        outs=[cc_out[:]],
        replica_groups=REPLICA_GROUPS,
    )
    # cc_out rank blocks are in group order (our own slot position varies per
    # core), so read back all 4 — our own block just overwrites identically.
    nc.scalar.dma_start(
        g4, cc_out[:].rearrange("(r p w) -> p r w", p=P, w=PKW)
    )

    # ---------------- sum the 4 rank partials ----------------
    t01 = chain.tile([P, PKW], F32, name="t01")
    t23 = chain.tile([P, PKW], F32, name="t23")
    gpk = chain.tile([P, PKW], F32, name="gpk")
    nc.vector.tensor_tensor(out=t01, in0=g4[:, 0], in1=g4[:, 1], op=OP.add)
    nc.vector.tensor_tensor(out=t23, in0=g4[:, 2], in1=g4[:, 3], op=OP.add)
    nc.vector.tensor_tensor(out=gpk, in0=t01, in1=t23, op=OP.add)

    # diag(G) (= per-channel sumsq) via identity mask + free-axis reduce
    dtmp = small.tile([P, CT, P], F32, name="dtmp")
    diag_h = small.tile([P, CT], F32, name="diag_h")
    for t in range(CT):
        nc.vector.tensor_mul(
            out=dtmp[:, t], in0=gpk[:, TRI_OFF[t]:TRI_OFF[t] + P], in1=ident
        )
        nc.vector.reduce_sum(diag_h[:, t:t + 1], dtmp[:, t], axis=AX.X)

    # ---------------- reconstruct full Gbar (fp16) from triangle ----------
    Gfull = chain.tile([P, CT, C], F16, name="Gfull")
    for t in range(CT):
        nc.scalar.copy(
            Gfull[:, t, t * P:C], gpk[:, TRI_OFF[t]:TRI_OFF[t] + TRI_W[t]]
        )
    tp_first = None
    for i in range(1, CT):
        tp_ps = ps_t.tile([P, i * P], F16, name=f"tp_ps{i}", tag="pt")
        for j in range(i):
            tp = nc.tensor.transpose(
                tp_ps[:, j * P:(j + 1) * P],
                Gfull[:, j, i * P:(i + 1) * P],
                ident_h,
            )
            if tp_first is None:
                tp_first = tp
        nc.scalar.copy(Gfull[:, i, 0:i * P], tp_ps)
    add_dep(tp_first.ins, dmy_last.ins, sync=False,
            reason="chain PE work queues after the keep-warm dummies")

    # ---------------- stats -> a, bvec ----------------
    sd_stack = small.tile([P, CT, 2], F32, name="sd_stack")
    for t in range(CT):
        nc.vector.tensor_copy(
            sd_stack[:, t, 0:1], gpk[:, S_COL[t]:S_COL[t] + 1]
        )
    nc.vector.tensor_copy(sd_stack[:, :, 1], diag_h)

    gsd = small.tile([P, CT, 2], F32, name="gsd")
    for ct in range(CT):
        gsd_ps = ps_t.tile([P, 2], F32, name=f"gsd_ps{ct}", tag="pt")
        gmm = nc.tensor.matmul(
            gsd_ps, adj, sd_stack[:, ct, :], start=True, stop=True
        )
        if ct == 0:
            add_dep(gmm.ins, dmy_last.ins, sync=False,
                    reason="chain PE work queues after the keep-warm dummies")
        nc.vector.tensor_copy(gsd[:, ct, :], gsd_ps)

    invN = 1.0 / float(GSIZE * N)
    meanex2 = small.tile([P, CT, 2], F32, name="meanex2")
    nc.vector.tensor_scalar_mul(meanex2, gsd, invN)
    mean = meanex2[:, :, 0]
    ex2 = meanex2[:, :, 1]
    msq = small.tile([P, CT], F32, name="msq")
    nc.vector.tensor_mul(out=msq, in0=mean, in1=mean)
    var = small.tile([P, CT], F32, name="var")
    nc.vector.scalar_tensor_tensor(
        out=var, in0=ex2, scalar=EPS, in1=msq, op0=OP.add, op1=OP.subtract
    )
    sd_ = small.tile([P, CT], F32, name="sd_")
    nc.scalar.sqrt(sd_, var)
    rstd = small.tile([P, CT], F32, name="rstd")
    nc.vector.reciprocal(rstd, sd_)
    a_sb = small.tile([P, CT], F32, name="a_sb")
    nc.vector.tensor_mul(out=a_sb, in0=rstd, in1=gw_sb)
    ma = small.tile([P, CT], F32, name="ma")
    nc.vector.tensor_mul(out=ma, in0=mean, in1=a_sb)
    bvec = small.tile([P, CT], F32, name="bvec")
    nc.vector.tensor_tensor(out=bvec, in0=gb_sb, in1=ma, op=OP.subtract)
    u1 = small.tile([P, CT], F32, name="u1")
    nc.vector.tensor_mul(out=u1, in0=a_sb, in1=sd_stack[:, :, 0])

    uv2 = small.tile([P, CT, 2], F16, name="uv2")
    nc.vector.tensor_copy(uv2[:, :, 0], u1)
    nc.vector.tensor_copy(uv2[:, :, 1], bvec)

    # ---------------- tq/bq, tk/bk (use UNscaled WqT/WkT) ----------------
    tb_q = small.tile([P, CT, 2], F32, name="tb_q")
    tb_k = small.tile([P, CT, 2], F32, name="tb_k")
    for j in range(CT):
        tb_ps = ps_t.tile([P, 2], F32, name=f"tbq_ps{j}", tag="pt")
        for ct in range(CT):
            nc.tensor.matmul(
                tb_ps,
                WqT[:, ct, j * P:(j + 1) * P],
                uv2[:, ct, :],
                start=(ct == 0),
                stop=(ct == CT - 1),
            )
        nc.vector.tensor_copy(tb_q[:, j, :], tb_ps)
        nc.vector.tensor_add(
            out=tb_q[:, j, 1:2], in0=tb_q[:, j, 1:2],
            in1=qkvb_sb[:, j:j + 1],
        )
    for j in range(CT):
        tb_ps = ps_t.tile([P, 2], F32, name=f"tbk_ps{j}", tag="pt")
        for ct in range(CT):
            nc.tensor.matmul(
                tb_ps,
                WkT[:, ct, j * P:(j + 1) * P],
                uv2[:, ct, :],
                start=(ct == 0),
                stop=(ct == CT - 1),
            )
        nc.vector.tensor_copy(tb_k[:, j, :], tb_ps)
        nc.vector.tensor_add(
            out=tb_k[:, j, 1:2], in0=tb_k[:, j, 1:2],
            in1=qkvb_sb[:, CT + j:CT + j + 1],
        )

    # scale WqT/WkT in place by a (per input-channel partition)
    for ct in range(CT):
        nc.vector.tensor_scalar_mul(
            WqT[:, ct, :], WqT[:, ct, :], a_sb[:, ct:ct + 1]
        )
        nc.vector.tensor_scalar_mul(
            WkT[:, ct, :], WkT[:, ct, :], a_sb[:, ct:ct + 1]
        )

    # wk2 = tk + N*bk
    wk2 = small.tile([P, CT], F32, name="wk2")
    nc.vector.tensor_scalar(wk2, tb_k[:, :, 1], float(N), None, OP.mult)
    nc.vector.tensor_add(out=wk2, in0=wk2, in1=tb_k[:, :, 0])

    # rank-1 padded operands (fp16 so they match the S matmul stream)
    Lpad = consts.tile([P, CT, P], F16, name="Lpad")
    Rpad = consts.tile([P, C], F16, name="Rpad")
    nc.vector.memset(Lpad, 0.0)
    nc.vector.memset(Rpad, 0.0)
    rstack = small.tile([P, CT, 2], F32, name="rstack")
    nc.vector.tensor_copy(rstack[:, :, 0], tb_k[:, :, 1])
    nc.vector.tensor_copy(rstack[:, :, 1], wk2)
    for j in range(CT):
        lt_ps = ps_t.tile([2, P], F32, name=f"lt_ps{j}", tag="pt")
        nc.tensor.transpose(lt_ps, tb_q[:, j, :], ident)
        nc.vector.tensor_copy(Lpad[0:2, j, :], lt_ps)
        rt_ps = ps_t.tile([2, P], F32, name=f"rt_ps{j}", tag="pt")
        nc.tensor.transpose(rt_ps, rstack[:, j, :], ident)
        nc.vector.tensor_copy(Rpad[0:2, j * P:(j + 1) * P], rt_ps)

    # ---------------- V = Gbar @ WkT_a   (fp16 x fp16) ----------------
    V_ps = [ps_g.tile([P, C], F32, name=f"V_ps{j}", tag="g") for j in range(CT)]
    for dt in range(CT):
        for j in range(CT):
            nc.tensor.matmul(
                V_ps[j],
                Gfull[:, dt, j * P:(j + 1) * P],
                WkT[:, dt, :],
                start=(dt == 0),
                stop=(dt == CT - 1),
            )
    V_sb = chain.tile([P, CT, C], F16, name="V_sb")
    for j in range(CT):
        nc.scalar.copy(V_sb[:, j, :], V_ps[j])

    # ------- S = WqT_a^T @ V + rank1 ; softmax ; R1 = attn^T @ PwT' -------
    # The softmax 1/rowsum is folded into PwT rows, so attn is the raw exp.
    # Software-pipelined: S matmuls for j+1 are issued before the R1 matmuls
    # of j so the PE is not stalled on softmax(j) latency.
    attn = chain.tile([P, CT, C], F16, name="attn")
    R1_ps = [ps_g.tile([P, C], F32, name=f"R1_ps{k}", tag="g")
             for k in range(CT)]
    for j in range(CT + 1):
        if j < CT:
            S_ps = ps_t.tile([P, C], F32, name=f"S_ps{j}", tag="pt")
            for ct in range(CT):
                nc.tensor.matmul(
                    S_ps,
                    WqT[:, ct, j * P:(j + 1) * P],
                    V_sb[:, ct, :],
                    start=(ct == 0),
                    stop=False,
                )
            nc.tensor.matmul(S_ps, Lpad[:, j, :], Rpad, start=False, stop=True)
            mx = small.tile([P, 1], F32, name=f"mx{j}")
            nc.vector.reduce_max(mx, S_ps, axis=AX.X)
            mb = small.tile([P, 1], F32, name=f"mb{j}")
            nc.vector.tensor_scalar_mul(mb, mx, -SCALE)
            rs = small.tile([P, 1], F32, name=f"rs{j}")
            nc.scalar.activation(
                attn[:, j, :], S_ps, ACTF.Exp,
                bias=mb, scale=SCALE, accum_out=rs,
            )
            rrec = small.tile([P, 1], F32, name=f"rrec{j}")
            nc.vector.reciprocal(rrec, rs)
            nc.vector.tensor_scalar_mul(PwT[:, j, :], PwT[:, j, :], rrec)
        if j > 0:
            for kb in range(CT):
                nc.tensor.matmul(
                    R1_ps[kb],
                    attn[:, j - 1, kb * P:(kb + 1) * P],
                    PwT[:, j - 1, :],
                    start=(j == 1),
                    stop=(j == CT),
                )

    R1_sb = chain.tile([P, CT, C], F16, name="R1_sb")
    for kb in range(CT):
        nc.scalar.copy(R1_sb[:, kb, :], R1_ps[kb])

    # d1 = (P attn) bv
    bvh = small.tile([P, CT], F16, name="bvh")
    nc.vector.tensor_copy(bvh, qkvb_sb[:, 2 * CT:3 * CT])
    d1 = small.tile([P, CT], F32, name="d1")
    for j in range(CT):
        d1_ps = ps_t.tile([P, 1], F32, name=f"d1_ps{j}", tag="pt")
        for kb in range(CT):
            nc.tensor.matmul(
                d1_ps,
                R1_sb[:, kb, j * P:(j + 1) * P],
                bvh[:, kb:kb + 1],
                start=(kb == 0),
                stop=(kb == CT - 1),
            )
        nc.vector.tensor_copy(d1[:, j:j + 1], d1_ps)

    # ---------------- R2 = Wv^T @ R1 ; A = diag(a) R2 + I ----------------
    R2_ps = [ps_g.tile([P, C], F32, name=f"R2_ps{i}", tag="g")
             for i in range(CT)]
    for kb in range(CT):
        for ib in range(CT):
            nc.tensor.matmul(
                R2_ps[ib],
                Wv_nat[:, kb, ib * P:(ib + 1) * P],
                R1_sb[:, kb, :],
                start=(kb == 0),
                stop=(kb == CT - 1),
            )
    A_h = consts.tile([P, CT, C], F16, name="A_h")
    for ib in range(CT):
        nc.vector.tensor_scalar_mul(
            A_h[:, ib, :], R2_ps[ib], a_sb[:, ib:ib + 1]
        )
        nc.vector.tensor_add(
            out=A_h[:, ib, ib * P:(ib + 1) * P],
            in0=A_h[:, ib, ib * P:(ib + 1) * P],
            in1=ident_h,
        )

    # d2 = R2^T bvec  (via A_h with bva = bvec/a; A includes +I)
    inv_a = small.tile([P, CT], F32, name="inv_a")
    nc.vector.reciprocal(inv_a, a_sb)
    bva = small.tile([P, CT], F32, name="bva")
    nc.vector.tensor_mul(out=bva, in0=bvec, in1=inv_a)
    bva_h = small.tile([P, CT], F16, name="bva_h")
    nc.vector.tensor_copy(bva_h, bva)
    d2 = small.tile([P, CT], F32, name="d2")
    for j in range(CT):
        d2_ps = ps_t.tile([P, 1], F32, name=f"d2_ps{j}", tag="pt")
        for ib in range(CT):
            nc.tensor.matmul(
                d2_ps,
                A_h[:, ib, j * P:(j + 1) * P],
                bva_h[:, ib:ib + 1],
                start=(ib == 0),
                stop=(ib == CT - 1),
            )
        nc.vector.tensor_copy(d2[:, j:j + 1], d2_ps)

    # delta = d1 + (d2 - bva) + proj_b
    delta = small.tile([P, CT], F32, name="delta")
    nc.vector.tensor_add(out=delta, in0=d1, in1=d2)
    nc.vector.tensor_tensor(out=delta, in0=delta, in1=bva, op=OP.subtract)
    nc.vector.tensor_add(out=delta, in0=delta, in1=pb_sb)

    # ------- phase 2: out = (I + A)^T x + delta  (fp16 stream + store) -----
    # x streams in 2048-token fp16 chunks (double-buffered); y is buffered
    # over 2048 tokens so the store DMA is 16KB-contiguous per partition.
    xn_dmas = []
    for v in range(N_STORES):
        xn_sb = xn_pool.tile([P, CT, PH2_STORE], F16, name=f"xn{v}", tag="xn")
        xn_dmas.append(nc.gpsimd.dma_start(xn_sb, xn_ext[:, v]))
        y_sb = y_pool.tile([P, CT, PH2_STORE], F16, name=f"y_sb{v}", tag="y")
        for w in range(SUB):
            u = v * SUB + w
            ysl = slice(w * PH2_CHUNK, (w + 1) * PH2_CHUNK)
            for j in range(CT):
                pool = ps_y if (u * CT + j) % 2 == 0 else ps_t
                tag = "y" if pool is ps_y else "pt"
                y_ps = pool.tile([P, PH2_CHUNK], F32,
                                 name=f"y_ps{u}_{j}", tag=tag)
                for ct in range(CT):
                    nc.tensor.matmul(
                        y_ps,
                        A_h[:, ct, j * P:(j + 1) * P],
                        xn_sb[:, ct, ysl],
                        start=(ct == 0),
                        stop=(ct == CT - 1),
                    )
                if j % 2 == 0:
                    nc.scalar.activation(
                        y_sb[:, j, ysl], y_ps, ACTF.Identity,
                        bias=delta[:, j:j + 1], scale=1.0,
                    )
                else:
                    nc.vector.tensor_scalar(
                        y_sb[:, j, ysl], y_ps, delta[:, j:j + 1], None, OP.add
                    )
        if v == N_STORES - 1:
            # split the final store so the tail after the last matmul is short
            half = PH2_STORE // 2
            nc.sync.dma_start(out_ext[:, v, :, 0:half], y_sb[:, :, 0:half])
            nc.sync.dma_start(out_ext[:, v, :, half:], y_sb[:, :, half:])
        else:
            nc.sync.dma_start(out_ext[:, v], y_sb)
    add_dep(xn_dmas[0].ins, xt_dmas[-1].ins, sync=True,
            reason="xn prefetch must not contend with the xt load for HBM")

    ctx.close()


_CACHED_NC = None


def _get_nc():
    global _CACHED_NC
    if _CACHED_NC is None:
        _CACHED_NC = build_graph()
    return _CACHED_NC


def make_in_maps(inputs):
    xf = np.asarray(inputs["x"], dtype=np.float32).reshape(B, C, N)
    qkv_w = np.asarray(inputs["qkv_w"], dtype=np.float32)
    proj_w = np.asarray(inputs["proj_w"], dtype=np.float32)

    # host-side weight transposes into [p, ct, c] block layout
    def blockT(w):  # w [co, ci] -> out[p, ct, co] = w[co, ct*128+p]
        return np.ascontiguousarray(w.T.reshape(CT, P, C).transpose(1, 0, 2))

    def blockN(w):  # w [co, ci] -> out[p, ct, ci] = w[ct*128+p, ci]
        return np.ascontiguousarray(w.reshape(CT, P, C).transpose(1, 0, 2))

    rep = {
        "wqt": blockT(qkv_w[0:C]).astype(np.float16),
        "wkt": blockT(qkv_w[C:2 * C]).astype(np.float16),
        "pwt": blockT(proj_w).astype(np.float16),
        "wv": blockN(qkv_w[2 * C:3 * C]).astype(np.float16),
        "ident": np.eye(P, dtype=np.float32),
        "gn_w": np.ascontiguousarray(np.asarray(inputs["gn_w"], np.float32)),
        "gn_b": np.ascontiguousarray(np.asarray(inputs["gn_b"], np.float32)),
        "qkv_b": np.ascontiguousarray(np.asarray(inputs["qkv_b"], np.float32)),
        "proj_b": np.ascontiguousarray(np.asarray(inputs["proj_b"], np.float32)),
    }
    ii = np.arange(P) // GSIZE
    rep["adjc"] = np.ascontiguousarray(
        (ii[:, None] == ii[None, :]).astype(np.float32)
    )

    in_maps = []
    for i in range(NCORES):
        b, sh = divmod(i, SHARDS)
        xsh = xf[b, :, sh * NS:(sh + 1) * NS]            # [C, NS] f32
        xsh_h = xsh.astype(np.float16)
        # token-major [p, u, c] + ones column at col C
        xt = np.zeros((P, PH1_ITERS, CPAD), dtype=np.float16)
        xt[:, :, 0:C] = xsh_h.T.reshape(PH1_ITERS, P, C).transpose(1, 0, 2)
        xt[:, :, C] = np.float16(1.0)
        # channel-major chunked [p, v, ct, tok]
        xn = np.ascontiguousarray(
            xsh_h.reshape(CT, P, N_STORES, PH2_STORE).transpose(1, 2, 0, 3)
        )
        m = {"xt": xt, "xn": xn}
        m.update(rep)
        in_maps.append(m)
    return in_maps


def assemble(results, inputs):
    x = np.asarray(inputs["x"])
    out = np.empty((B, C, N), dtype=np.float32)
    for i in range(NCORES):
        b, sh = divmod(i, SHARDS)
        # res [p, v, ct, tok] -> [C, NS]
        res = np.asarray(results[i]["out"], dtype=np.float32)
        out[b, :, sh * NS:(sh + 1) * NS] = (
            res.transpose(2, 0, 1, 3).reshape(C, NS)
        )
    return out.reshape(x.shape)


def kernel(**inputs) -> np.ndarray:
    nc = _get_nc()
    res = run_bass_kernel_spmd(nc, make_in_maps(inputs), list(range(NCORES)))
    return assemble(res.results, inputs)


if __name__ == "__main__":
    # quick smoke: build only
    build_graph()
    print("build OK")


# revision 15
# speedup vs baseline: 1.0587x; 1.0587x over previous
"""Distributed Trainium2 kernel for nn_AttentionBlock (channel attention).

Algorithm (exact algebra, no approximation):
  The attention matrix is [C,C] with the contraction over N=H*W*D tokens.
  GroupNorm is a per-channel affine xn = a*x + b whose stats derive from
  per-channel sums s = x@1 and the Gram matrix G = x@x.T (diag(G) = sumsq).
  Everything downstream of G is [C,C]-sized:
      S    = Wq' G Wk'^T + rank-1 terms        (Wq' = Wq diag(a))
      attn = softmax(S/sqrt(C))
      out  = x + P attn Wv' x + delta 1^T
  Pass 1 computes only the upper-triangle blocks of G (G is symmetric),
  with per-channel sums riding as a ones column in the same matmuls.
  The host pre-transposes/pre-casts x into fp16 token-major tiles so pass 1
  is pure matmul (no on-chip transposes), and pre-transposes the weights.
  The [C,C] partial results cross cores via an fp16 AllGather (the f32
  AllReduce ran at ~20GB/s bus here; a gather of the fp16 triangle + 3 DVE
  adds is ~4x faster), then the [C,C] chain and a streamed pass 2
      out = (I + A)^T x + delta
  with fp16 chunk loads and fully-contiguous fp16 block stores.

Sharding: batch B=2 x sequence 4  ->  8 cores. replica groups [[0..3],[4..7]].
"""

from contextlib import ExitStack

import numpy as np

import concourse.bass as bass
import concourse.tile as tile
from concourse import bacc, mybir
from concourse.bass_utils import run_bass_kernel_spmd
from concourse.bass import _add_dep_helper as add_dep

# Problem constants (hardcoded per harness contract)
B = 2
C = 512
N = 32768          # 32*32*32
NCORES = 8
SHARDS = 4         # sequence shards per batch
NS = N // SHARDS   # 8192 per-core tokens
GROUPS = 32
GSIZE = C // GROUPS  # 16
EPS = 1e-5
P = 128
CT = C // P        # 4 channel tiles
F32 = mybir.dt.float32
F16 = mybir.dt.float16

PH1_ITERS = NS // P             # 64 token chunks of 128
CPAD = C + 4                    # xt row: 512 cols + ones col + pad
XT_DMAS = 16
XT_STEP = PH1_ITERS // XT_DMAS  # 4 chunks per DMA
N_DUMMY = 120                   # PE keep-warm matmuls during the exchange

PH2_CHUNK = 512
PH2_STORE = 2048
N_STORES = NS // PH2_STORE      # 4
SUB = PH2_STORE // PH2_CHUNK    # 4

# triangle packing: row-block t holds blocks (t, t..3); blocks 1..3 carry a
# trailing s column (from the ones-col matmul); s0 is packed last.
TRI_W = [C - t * P for t in range(CT)]          # 512,384,256,128
TRI_OFF = [0, 512, 897, 1154]
PKW = 1284                                       # 512+385+257+129+1
S_COL = [1283, 896, 1153, 1282]                  # s cols for blocks 0..3

REPLICA_GROUPS = [[0, 1, 2, 3], [4, 5, 6, 7]]
SCALE = 1.0 / float(np.sqrt(C))


def build_graph():
    nc = bacc.Bacc(
        "TRN2", target_bir_lowering=False, debug=False, num_devices=NCORES
    )

    xt_ext = nc.dram_tensor("xt", [P, PH1_ITERS, CPAD], F16, kind="ExternalInput")
    xn_ext = nc.dram_tensor("xn", [P, N_STORES, CT, PH2_STORE], F16,
                            kind="ExternalInput")
    wqt_ext = nc.dram_tensor("wqt", [P, CT, C], F16, kind="ExternalInput")
    wkt_ext = nc.dram_tensor("wkt", [P, CT, C], F16, kind="ExternalInput")
    pwt_ext = nc.dram_tensor("pwt", [P, CT, C], F16, kind="ExternalInput")
    wv_ext = nc.dram_tensor("wv", [P, CT, C], F16, kind="ExternalInput")
    ident_ext = nc.dram_tensor("ident", [P, P], F32, kind="ExternalInput")
    adjc_ext = nc.dram_tensor("adjc", [P, P], F32, kind="ExternalInput")
    gn_w_ext = nc.dram_tensor("gn_w", [C], F32, kind="ExternalInput")
    gn_b_ext = nc.dram_tensor("gn_b", [C], F32, kind="ExternalInput")
    qkv_b_ext = nc.dram_tensor("qkv_b", [3 * C], F32, kind="ExternalInput")
    proj_b_ext = nc.dram_tensor("proj_b", [C], F32, kind="ExternalInput")
    out_ext = nc.dram_tensor("out", [P, N_STORES, CT, PH2_STORE], F16,
                             kind="ExternalOutput")

    with tile.TileContext(nc) as tc:
        _body(tc, xt_ext, xn_ext, wqt_ext, wkt_ext, pwt_ext, wv_ext,
              ident_ext, adjc_ext, gn_w_ext, gn_b_ext, qkv_b_ext,
              proj_b_ext, out_ext)

    nc.compile()
    return nc


def _body(tc, xt_ext, xn_ext, wqt_ext, wkt_ext, pwt_ext, wv_ext,
          ident_ext, adjc_ext, gn_w_ext, gn_b_ext, qkv_b_ext,
          proj_b_ext, out_ext):
    nc = tc.nc
    AX = mybir.AxisListType
    OP = mybir.AluOpType
    ACTF = mybir.ActivationFunctionType

    ctx = ExitStack()
    consts = ctx.enter_context(tc.tile_pool(name="consts", bufs=1))
    small = ctx.enter_context(tc.tile_pool(name="small", bufs=1))
    wpool = ctx.enter_context(tc.tile_pool(name="wpool", bufs=1))
    xt_pool = ctx.enter_context(tc.tile_pool(name="xt", bufs=1))
    chain = ctx.enter_context(tc.tile_pool(name="chain", bufs=1))
    xn_pool = ctx.enter_context(tc.tile_pool(name="xn", bufs=2))
    y_pool = ctx.enter_context(tc.tile_pool(name="yp", bufs=2))
    ps_g = ctx.enter_context(tc.tile_pool(name="psg", bufs=4, space="PSUM"))
    ps_t = ctx.enter_context(tc.tile_pool(name="pst", bufs=2, space="PSUM"))
    ps_y = ctx.enter_context(tc.tile_pool(name="psy", bufs=2, space="PSUM"))
    dram = ctx.enter_context(tc.tile_pool(name="dram", bufs=1, space="DRAM"))

    # ---------------- x token-major load (paces phase 1; full HWDGE rate,
    # nothing else touches HBM until these are done) ----------------
    xt_sb = xt_pool.tile([P, PH1_ITERS, CPAD], F16, name="xt_sb")
    xt_dmas = []
    for m in range(XT_DMAS):
        sl = slice(m * XT_STEP, (m + 1) * XT_STEP)
        xt_dmas.append(nc.sync.dma_start(xt_sb[:, sl], xt_ext[:, sl]))

    # small constants on the scalar (ACT) HWDGE ring
    ident = consts.tile([P, P], F32, name="ident")
    nc.scalar.dma_start(ident, ident_ext[:])
    adj = consts.tile([P, P], F32, name="adj")          # 16x16 block-diag ones
    nc.scalar.dma_start(adj, adjc_ext[:])
    gw_sb = consts.tile([P, CT], F32, name="gw_sb")
    gb_sb = consts.tile([P, CT], F32, name="gb_sb")
    pb_sb = consts.tile([P, CT], F32, name="pb_sb")
    nc.scalar.dma_start(gw_sb, gn_w_ext[:].rearrange("(t p) -> p t", p=P))
    nc.scalar.dma_start(gb_sb, gn_b_ext[:].rearrange("(t p) -> p t", p=P))
    nc.scalar.dma_start(pb_sb, proj_b_ext[:].rearrange("(t p) -> p t", p=P))
    qkvb_sb = consts.tile([P, 3 * CT], F32, name="qkvb_sb")
    nc.scalar.dma_start(qkvb_sb, qkv_b_ext[:].rearrange("(t p) -> p t", p=P))

    ident_h = consts.tile([P, P], F16, name="ident_h")
    nc.vector.tensor_copy(ident_h, ident)

    # ------- weights (pre-transposed on host; load after xt for HBM priority)
    WqT = wpool.tile([P, CT, C], F16, name="WqT")
    WkT = wpool.tile([P, CT, C], F16, name="WkT")
    PwT = wpool.tile([P, CT, C], F16, name="PwT")
    Wv_nat = wpool.tile([P, CT, C], F16, name="Wv_nat")
    w_dmas = []
    for Wdst, src in ((WqT, wqt_ext), (WkT, wkt_ext),
                      (PwT, pwt_ext), (Wv_nat, wv_ext)):
        w_dmas.append(nc.sync.dma_start(Wdst, src[:]))
    add_dep(w_dmas[0].ins, xt_dmas[-1].ins, sync=True,
            reason="weight loads must not contend with the xt load for HBM")

    # ------- phase 1: upper-triangle G blocks (fp16), s = x @ 1 -------
    # xt is token-major from the host: no transposes, just matmuls. The ones
    # column at xt col C makes s ride along for blocks 1..3; block 0's s
    # comes from a 1-col matmul reusing the block-0 stationary operand.
    G_ps = [ps_g.tile([P, TRI_W[t] + (1 if t > 0 else 0)], F32,
                      name=f"G_ps{t}", tag="g")
            for t in range(CT)]
    s0_ps = ps_t.tile([P, 1], F32, name="s0_ps", tag="pt")
    g_last = None
    for u in range(PH1_ITERS):
        xu = xt_sb[:, u]
        st, sp = (u == 0), (u == PH1_ITERS - 1)
        for t in range(CT):
            nc.tensor.matmul(
                G_ps[t],
                xu[:, t * P:(t + 1) * P],
                xu[:, t * P:C + (1 if t > 0 else 0)],
                start=st, stop=sp,
            )
        g_last = nc.tensor.matmul(
            s0_ps, xu[:, 0:P], xu[:, C:C + 1], start=st, stop=sp
        )

    # ---- pack fp16 payload [T0 | T1+s1 | T2+s2 | T3+s3 | s0] into our own
    # slot of the gather buffer ----
    g4 = chain.tile([P, SHARDS, PKW], F16, name="g4")
    pk = g4[:, 0]
    nc.scalar.copy(pk[:, 0:TRI_W[0]], G_ps[0])
    nc.vector.tensor_copy(pk[:, TRI_OFF[1]:TRI_OFF[1] + TRI_W[1] + 1], G_ps[1])
    nc.scalar.copy(pk[:, TRI_OFF[2]:TRI_OFF[2] + TRI_W[2] + 1], G_ps[2])
    nc.vector.tensor_copy(pk[:, TRI_OFF[3]:TRI_OFF[3] + TRI_W[3] + 1], G_ps[3])
    nc.vector.tensor_copy(pk[:, PKW - 1:PKW], s0_ps)

    # ------- PE keep-warm dummies: the HAM clock gate re-throttles the PE to
    # 1.2GHz after ~3.4us idle; the exchange window would otherwise make the
    # whole [C,C] chain run cold. Pinned after phase 1 on the PE queue. ------
    dummy_ps = ps_y.tile([P, C], F32, name="dummy_ps", tag="y")
    dmy_first = dmy_last = None
    for k in range(N_DUMMY):
        mm = nc.tensor.matmul(
            dummy_ps, xt_sb[:, 0, 0:P], xt_sb[:, 0, 0:C],
            start=True, stop=True,
        )
        if dmy_first is None:
            dmy_first = mm
        dmy_last = mm
    add_dep(dmy_first.ins, g_last.ins, sync=False,
            reason="dummies run after phase 1 in PE queue order")

    # ------- fp16 AllGather of the packed triangle (the f32 AllReduce ran at
    # 19.8GB/s bus; the gather moves half the bytes and skips the CCE reduce).
    # A hand-rolled remote_dma exchange was tried and is ~5x faster on paper,
    # but delivery is non-deterministic under this runtime shim — reverted.
    cc_in = dram.tile([P * PKW], F16, name="cc_in")
    cc_out = dram.tile([SHARDS * P * PKW], F16, name="cc_out")
    cc_iv = cc_in[:].rearrange("(p w) -> p w", p=P)
    # bounce-in in pieces so the DMA pipelines with the pack copies
    nc.scalar.dma_start(cc_iv[:, 0:TRI_OFF[2]], pk[:, 0:TRI_OFF[2]])
    bounce_last = nc.scalar.dma_start(cc_iv[:, TRI_OFF[2]:PKW],
                                      pk[:, TRI_OFF[2]:PKW])

    nc.gpsimd.collective_compute(
        "AllGather",
        OP.bypass,
        ins=[cc_in[:]],
        outs=[cc_out[:]],
        replica_groups=REPLICA_GROUPS,
    )
    # cc_out rank blocks are in group order (our own slot position varies per
    # core), so read back all 4 — our own block just overwrites identically.
    # Two pieces so the partial sums pipeline with the readback DMA.
    cc_ov = cc_out[:].rearrange("(r p w) -> p r w", p=P, w=PKW)
    SPLIT = TRI_OFF[2]
    nc.scalar.dma_start(g4[:, :, 0:SPLIT], cc_ov[:, :, 0:SPLIT])
    nc.scalar.dma_start(g4[:, :, SPLIT:PKW], cc_ov[:, :, SPLIT:PKW])

    # ---------------- sum the 4 rank partials (piecewise) ----------------
    t01 = chain.tile([P, PKW], F32, name="t01")
    t23 = chain.tile([P, PKW], F32, name="t23")
    gpk = chain.tile([P, PKW], F32, name="gpk")
    for a, b in ((0, SPLIT), (SPLIT, PKW)):
        nc.vector.tensor_tensor(out=t01[:, a:b], in0=g4[:, 0, a:b],
                                in1=g4[:, 1, a:b], op=OP.add)
        nc.vector.tensor_tensor(out=t23[:, a:b], in0=g4[:, 2, a:b],
                                in1=g4[:, 3, a:b], op=OP.add)
        nc.vector.tensor_tensor(out=gpk[:, a:b], in0=t01[:, a:b],
                                in1=t23[:, a:b], op=OP.add)

    # diag(G) (= per-channel sumsq) via identity mask + free-axis reduce
    dtmp = small.tile([P, CT, P], F32, name="dtmp")
    diag_h = small.tile([P, CT], F32, name="diag_h")
    for t in range(CT):
        nc.vector.tensor_mul(
            out=dtmp[:, t], in0=gpk[:, TRI_OFF[t]:TRI_OFF[t] + P], in1=ident
        )
        nc.vector.reduce_sum(diag_h[:, t:t + 1], dtmp[:, t], axis=AX.X)

    # ---------------- reconstruct full Gbar (fp16) from triangle ----------
    Gfull = chain.tile([P, CT, C], F16, name="Gfull")
    for t in range(CT):
        nc.scalar.copy(
            Gfull[:, t, t * P:C], gpk[:, TRI_OFF[t]:TRI_OFF[t] + TRI_W[t]]
        )
    tp_first = None
    for i in range(1, CT):
        tp_ps = ps_t.tile([P, i * P], F16, name=f"tp_ps{i}", tag="pt")
        for j in range(i):
            tp = nc.tensor.transpose(
                tp_ps[:, j * P:(j + 1) * P],
                Gfull[:, j, i * P:(i + 1) * P],
                ident_h,
            )
            if tp_first is None:
                tp_first = tp
        nc.scalar.copy(Gfull[:, i, 0:i * P], tp_ps)
    add_dep(tp_first.ins, dmy_last.ins, sync=False,
            reason="chain PE work queues after the keep-warm dummies")

    # ---------------- stats -> a, bvec ----------------
    sd_stack = small.tile([P, CT, 2], F32, name="sd_stack")
    for t in range(CT):
        nc.vector.tensor_copy(
            sd_stack[:, t, 0:1], gpk[:, S_COL[t]:S_COL[t] + 1]
        )
    nc.vector.tensor_copy(sd_stack[:, :, 1], diag_h)

    gsd = small.tile([P, CT, 2], F32, name="gsd")
    for ct in range(CT):
        gsd_ps = ps_t.tile([P, 2], F32, name=f"gsd_ps{ct}", tag="pt")
        gmm = nc.tensor.matmul(
            gsd_ps, adj, sd_stack[:, ct, :], start=True, stop=True
        )
        if ct == 0:
            add_dep(gmm.ins, dmy_last.ins, sync=False,
                    reason="chain PE work queues after the keep-warm dummies")
        nc.vector.tensor_copy(gsd[:, ct, :], gsd_ps)

    invN = 1.0 / float(GSIZE * N)
    meanex2 = small.tile([P, CT, 2], F32, name="meanex2")
    nc.vector.tensor_scalar_mul(meanex2, gsd, invN)
    mean = meanex2[:, :, 0]
    ex2 = meanex2[:, :, 1]
    msq = small.tile([P, CT], F32, name="msq")
    nc.vector.tensor_mul(out=msq, in0=mean, in1=mean)
    var = small.tile([P, CT], F32, name="var")
    nc.vector.scalar_tensor_tensor(
        out=var, in0=ex2, scalar=EPS, in1=msq, op0=OP.add, op1=OP.subtract
    )
    sd_ = small.tile([P, CT], F32, name="sd_")
    nc.scalar.sqrt(sd_, var)
    rstd = small.tile([P, CT], F32, name="rstd")
    nc.vector.reciprocal(rstd, sd_)
    a_sb = small.tile([P, CT], F32, name="a_sb")
    nc.vector.tensor_mul(out=a_sb, in0=rstd, in1=gw_sb)
    ma = small.tile([P, CT], F32, name="ma")
    nc.vector.tensor_mul(out=ma, in0=mean, in1=a_sb)
    bvec = small.tile([P, CT], F32, name="bvec")
    nc.vector.tensor_tensor(out=bvec, in0=gb_sb, in1=ma, op=OP.subtract)
    u1 = small.tile([P, CT], F32, name="u1")
    nc.vector.tensor_mul(out=u1, in0=a_sb, in1=sd_stack[:, :, 0])

    uv2 = small.tile([P, CT, 2], F16, name="uv2")
    nc.vector.tensor_copy(uv2[:, :, 0], u1)
    nc.vector.tensor_copy(uv2[:, :, 1], bvec)

    # ---------------- tq/bq, tk/bk (use UNscaled WqT/WkT) ----------------
    tb_q = small.tile([P, CT, 2], F32, name="tb_q")
    tb_k = small.tile([P, CT, 2], F32, name="tb_k")
    for j in range(CT):
        tb_ps = ps_t.tile([P, 2], F32, name=f"tbq_ps{j}", tag="pt")
        for ct in range(CT):
            nc.tensor.matmul(
                tb_ps,
                WqT[:, ct, j * P:(j + 1) * P],
                uv2[:, ct, :],
                start=(ct == 0),
                stop=(ct == CT - 1),
            )
        nc.vector.tensor_copy(tb_q[:, j, :], tb_ps)
        nc.vector.tensor_add(
            out=tb_q[:, j, 1:2], in0=tb_q[:, j, 1:2],
            in1=qkvb_sb[:, j:j + 1],
        )
    for j in range(CT):
        tb_ps = ps_t.tile([P, 2], F32, name=f"tbk_ps{j}", tag="pt")
        for ct in range(CT):
            nc.tensor.matmul(
                tb_ps,
                WkT[:, ct, j * P:(j + 1) * P],
                uv2[:, ct, :],
                start=(ct == 0),
                stop=(ct == CT - 1),
            )
        nc.vector.tensor_copy(tb_k[:, j, :], tb_ps)
        nc.vector.tensor_add(
            out=tb_k[:, j, 1:2], in0=tb_k[:, j, 1:2],
            in1=qkvb_sb[:, CT + j:CT + j + 1],
        )

    # scale WqT/WkT in place by a (per input-channel partition)
    for ct in range(CT):
        nc.vector.tensor_scalar_mul(
            WqT[:, ct, :], WqT[:, ct, :], a_sb[:, ct:ct + 1]
        )
        nc.vector.tensor_scalar_mul(
            WkT[:, ct, :], WkT[:, ct, :], a_sb[:, ct:ct + 1]
        )

    # wk2 = tk + N*bk
    wk2 = small.tile([P, CT], F32, name="wk2")
    nc.vector.tensor_scalar(wk2, tb_k[:, :, 1], float(N), None, OP.mult)
    nc.vector.tensor_add(out=wk2, in0=wk2, in1=tb_k[:, :, 0])

    # rank-1 padded operands (fp16 so they match the S matmul stream)
    Lpad = consts.tile([P, CT, P], F16, name="Lpad")
    Rpad = consts.tile([P, C], F16, name="Rpad")
    nc.vector.memset(Lpad, 0.0)
    nc.vector.memset(Rpad, 0.0)
    rstack = small.tile([P, CT, 2], F32, name="rstack")
    nc.vector.tensor_copy(rstack[:, :, 0], tb_k[:, :, 1])
    nc.vector.tensor_copy(rstack[:, :, 1], wk2)
    for j in range(CT):
        lt_ps = ps_t.tile([2, P], F32, name=f"lt_ps{j}", tag="pt")
        nc.tensor.transpose(lt_ps, tb_q[:, j, :], ident)
        nc.vector.tensor_copy(Lpad[0:2, j, :], lt_ps)
        rt_ps = ps_t.tile([2, P], F32, name=f"rt_ps{j}", tag="pt")
        nc.tensor.transpose(rt_ps, rstack[:, j, :], ident)
        nc.vector.tensor_copy(Rpad[0:2, j * P:(j + 1) * P], rt_ps)

    # ---------------- V = Gbar @ WkT_a   (fp16 x fp16) ----------------
    V_ps = [ps_g.tile([P, C], F32, name=f"V_ps{j}", tag="g") for j in range(CT)]
    for dt in range(CT):
        for j in range(CT):
            nc.tensor.matmul(
                V_ps[j],
                Gfull[:, dt, j * P:(j + 1) * P],
                WkT[:, dt, :],
                start=(dt == 0),
                stop=(dt == CT - 1),
            )
    V_sb = chain.tile([P, CT, C], F16, name="V_sb")
    for j in range(CT):
        nc.scalar.copy(V_sb[:, j, :], V_ps[j])

    # ------- S = WqT_a^T @ V + rank1 ; softmax ; R1 = attn^T @ PwT' -------
    # The softmax 1/rowsum is folded into PwT rows, so attn is the raw exp.
    # Software-pipelined: S matmuls for j+1 are issued before the R1 matmuls
    # of j so the PE is not stalled on softmax(j) latency.
    attn = chain.tile([P, CT, C], F16, name="attn")
    R1_ps = [ps_g.tile([P, C], F32, name=f"R1_ps{k}", tag="g")
             for k in range(CT)]
    for j in range(CT + 1):
        if j < CT:
            S_ps = ps_t.tile([P, C], F32, name=f"S_ps{j}", tag="pt")
            for ct in range(CT):
                nc.tensor.matmul(
                    S_ps,
                    WqT[:, ct, j * P:(j + 1) * P],
                    V_sb[:, ct, :],
                    start=(ct == 0),
                    stop=False,
                )
            nc.tensor.matmul(S_ps, Lpad[:, j, :], Rpad, start=False, stop=True)
            mx = small.tile([P, 1], F32, name=f"mx{j}")
            nc.vector.reduce_max(mx, S_ps, axis=AX.X)
            mb = small.tile([P, 1], F32, name=f"mb{j}")
            nc.vector.tensor_scalar_mul(mb, mx, -SCALE)
            rs = small.tile([P, 1], F32, name=f"rs{j}")
            nc.scalar.activation(
                attn[:, j, :], S_ps, ACTF.Exp,
                bias=mb, scale=SCALE, accum_out=rs,
            )
            rrec = small.tile([P, 1], F32, name=f"rrec{j}")
            nc.vector.reciprocal(rrec, rs)
            nc.vector.tensor_scalar_mul(PwT[:, j, :], PwT[:, j, :], rrec)
        if j > 0:
            for kb in range(CT):
                nc.tensor.matmul(
                    R1_ps[kb],
                    attn[:, j - 1, kb * P:(kb + 1) * P],
                    PwT[:, j - 1, :],
                    start=(j == 1),
                    stop=(j == CT),
                )

    R1_sb = chain.tile([P, CT, C], F16, name="R1_sb")
    for kb in range(CT):
        nc.scalar.copy(R1_sb[:, kb, :], R1_ps[kb])

    # d1 = (P attn) bv
    bvh = small.tile([P, CT], F16, name="bvh")
    nc.vector.tensor_copy(bvh, qkvb_sb[:, 2 * CT:3 * CT])
    d1 = small.tile([P, CT], F32, name="d1")
    for j in range(CT):
        d1_ps = ps_t.tile([P, 1], F32, name=f"d1_ps{j}", tag="pt")
        for kb in range(CT):
            nc.tensor.matmul(
                d1_ps,
                R1_sb[:, kb, j * P:(j + 1) * P],
                bvh[:, kb:kb + 1],
                start=(kb == 0),
                stop=(kb == CT - 1),
            )
        nc.vector.tensor_copy(d1[:, j:j + 1], d1_ps)

    # ---------------- R2 = Wv^T @ R1 ; A = diag(a) R2 + I ----------------
    R2_ps = [ps_g.tile([P, C], F32, name=f"R2_ps{i}", tag="g")
             for i in range(CT)]
    for kb in range(CT):
        for ib in range(CT):
            nc.tensor.matmul(
                R2_ps[ib],
                Wv_nat[:, kb, ib * P:(ib + 1) * P],
                R1_sb[:, kb, :],
                start=(kb == 0),
                stop=(kb == CT - 1),
            )
    A_h = consts.tile([P, CT, C], F16, name="A_h")
    for ib in range(CT):
        nc.vector.tensor_scalar_mul(
            A_h[:, ib, :], R2_ps[ib], a_sb[:, ib:ib + 1]
        )
        nc.vector.tensor_add(
            out=A_h[:, ib, ib * P:(ib + 1) * P],
            in0=A_h[:, ib, ib * P:(ib + 1) * P],
            in1=ident_h,
        )

    # d2 = R2^T bvec  (via A_h with bva = bvec/a; A includes +I)
    inv_a = small.tile([P, CT], F32, name="inv_a")
    nc.vector.reciprocal(inv_a, a_sb)
    bva = small.tile([P, CT], F32, name="bva")
    nc.vector.tensor_mul(out=bva, in0=bvec, in1=inv_a)
    bva_h = small.tile([P, CT], F16, name="bva_h")
    nc.vector.tensor_copy(bva_h, bva)
    d2 = small.tile([P, CT], F32, name="d2")
    for j in range(CT):
        d2_ps = ps_t.tile([P, 1], F32, name=f"d2_ps{j}", tag="pt")
        for ib in range(CT):
            nc.tensor.matmul(
                d2_ps,
                A_h[:, ib, j * P:(j + 1) * P],
                bva_h[:, ib:ib + 1],
                start=(ib == 0),
                stop=(ib == CT - 1),
            )
        nc.vector.tensor_copy(d2[:, j:j + 1], d2_ps)

    # delta = d1 + (d2 - bva) + proj_b
    delta = small.tile([P, CT], F32, name="delta")
    nc.vector.tensor_add(out=delta, in0=d1, in1=d2)
    nc.vector.tensor_tensor(out=delta, in0=delta, in1=bva, op=OP.subtract)
    nc.vector.tensor_add(out=delta, in0=delta, in1=pb_sb)

    # ------- phase 2: out = (I + A)^T x + delta  (fp16 stream + store) -----
    # x streams in 2048-token fp16 chunks (double-buffered); y is buffered
    # over 2048 tokens so the store DMA is 16KB-contiguous per partition.
    xn_dmas = []
    for v in range(N_STORES):
        xn_sb = xn_pool.tile([P, CT, PH2_STORE], F16, name=f"xn{v}", tag="xn")
        xn_dmas.append(nc.gpsimd.dma_start(xn_sb, xn_ext[:, v]))
        y_sb = y_pool.tile([P, CT, PH2_STORE], F16, name=f"y_sb{v}", tag="y")
        for w in range(SUB):
            u = v * SUB + w
            ysl = slice(w * PH2_CHUNK, (w + 1) * PH2_CHUNK)
            for j in range(CT):
                pool = ps_y if (u * CT + j) % 2 == 0 else ps_t
                tag = "y" if pool is ps_y else "pt"
                y_ps = pool.tile([P, PH2_CHUNK], F32,
                                 name=f"y_ps{u}_{j}", tag=tag)
                for ct in range(CT):
                    nc.tensor.matmul(
                        y_ps,
                        A_h[:, ct, j * P:(j + 1) * P],
                        xn_sb[:, ct, ysl],
                        start=(ct == 0),
                        stop=(ct == CT - 1),
                    )
                if j % 2 == 0:
                    nc.scalar.activation(
                        y_sb[:, j, ysl], y_ps, ACTF.Identity,
                        bias=delta[:, j:j + 1], scale=1.0,
                    )
                else:
                    nc.vector.tensor_scalar(
                        y_sb[:, j, ysl], y_ps, delta[:, j:j + 1], None, OP.add
                    )
        if v == N_STORES - 1:
            # split the final store so the tail after the last matmul is short
            half = PH2_STORE // 2
            nc.sync.dma_start(out_ext[:, v, :, 0:half], y_sb[:, :, 0:half])
            nc.sync.dma_start(out_ext[:, v, :, half:], y_sb[:, :, half:])
        else:
            nc.sync.dma_start(out_ext[:, v], y_sb)
    # the xn prefetch shares the 16 SDMA engines with everything else; gate it
    # behind the collective bounce-in so it cannot delay the AG trigger
    # (observed +9us when 4MB of xn traffic straddled the bounce window)
    add_dep(xn_dmas[0].ins, bounce_last.ins, sync=True,
            reason="xn prefetch must not delay the collective bounce-in")
    add_dep(xn_dmas[1].ins, bounce_last.ins, sync=True,
            reason="xn prefetch must not delay the collective bounce-in")

    ctx.close()


_CACHED_NC = None


def _get_nc():
    global _CACHED_NC
    if _CACHED_NC is None:
        _CACHED_NC = build_graph()
    return _CACHED_NC


def make_in_maps(inputs):
    xf = np.asarray(inputs["x"], dtype=np.float32).reshape(B, C, N)
    qkv_w = np.asarray(inputs["qkv_w"], dtype=np.float32)
    proj_w = np.asarray(inputs["proj_w"], dtype=np.float32)

    # host-side weight transposes into [p, ct, c] block layout
    def blockT(w):  # w [co, ci] -> out[p, ct, co] = w[co, ct*128+p]
        return np.ascontiguousarray(w.T.reshape(CT, P, C).transpose(1, 0, 2))

    def blockN(w):  # w [co, ci] -> out[p, ct, ci] = w[ct*128+p, ci]
        return np.ascontiguousarray(w.reshape(CT, P, C).transpose(1, 0, 2))

    rep = {
        "wqt": blockT(qkv_w[0:C]).astype(np.float16),
        "wkt": blockT(qkv_w[C:2 * C]).astype(np.float16),
        "pwt": blockT(proj_w).astype(np.float16),
        "wv": blockN(qkv_w[2 * C:3 * C]).astype(np.float16),
        "ident": np.eye(P, dtype=np.float32),
        "gn_w": np.ascontiguousarray(np.asarray(inputs["gn_w"], np.float32)),
        "gn_b": np.ascontiguousarray(np.asarray(inputs["gn_b"], np.float32)),
        "qkv_b": np.ascontiguousarray(np.asarray(inputs["qkv_b"], np.float32)),
        "proj_b": np.ascontiguousarray(np.asarray(inputs["proj_b"], np.float32)),
    }
    ii = np.arange(P) // GSIZE
    rep["adjc"] = np.ascontiguousarray(
        (ii[:, None] == ii[None, :]).astype(np.float32)
    )

    in_maps = []
    for i in range(NCORES):
        b, sh = divmod(i, SHARDS)
        xsh = xf[b, :, sh * NS:(sh + 1) * NS]            # [C, NS] f32
        xsh_h = xsh.astype(np.float16)
        # token-major [p, u, c] + ones column at col C
        xt = np.zeros((P, PH1_ITERS, CPAD), dtype=np.float16)
        xt[:, :, 0:C] = xsh_h.T.reshape(PH1_ITERS, P, C).transpose(1, 0, 2)
        xt[:, :, C] = np.float16(1.0)
        # channel-major chunked [p, v, ct, tok]
        xn = np.ascontiguousarray(
            xsh_h.reshape(CT, P, N_STORES, PH2_STORE).transpose(1, 2, 0, 3)
        )
        m = {"xt": xt, "xn": xn}
        m.update(rep)
        in_maps.append(m)
    return in_maps


def assemble(results, inputs):
    x = np.asarray(inputs["x"])
    out = np.empty((B, C, N), dtype=np.float32)
    for i in range(NCORES):
        b, sh = divmod(i, SHARDS)
        # res [p, v, ct, tok] -> [C, NS]
        res = np.asarray(results[i]["out"], dtype=np.float32)
        out[b, :, sh * NS:(sh + 1) * NS] = (
            res.transpose(2, 0, 1, 3).reshape(C, NS)
        )
    return out.reshape(x.shape)


def kernel(**inputs) -> np.ndarray:
    nc = _get_nc()
    res = run_bass_kernel_spmd(nc, make_in_maps(inputs), list(range(NCORES)))
    return assemble(res.results, inputs)


if __name__ == "__main__":
    # quick smoke: build only
    build_graph()
    print("build OK")
